# revision 2
# baseline (speedup 1.0000x reference)
"""Causal depthwise Conv1d (K=4) for Trainium2, 8 NeuronCores — v2.

Problem: x (B=8, L=4096, D=1024) f32, w (D, 1, 4), b (D,)
  y[n, l, d] = sum_k w[d, 0, k] * x[n, l - 3 + k, d] + b[d]   (zero pad l<0)

Sharding: data-parallel over batch — core i computes batch item i.

v2 strategy (vs v1's on-device PE transposes + fp32 exact):
  * Host pre-transposes x to channels-major (D, L) and casts to fp16;
    device I/O is fp16 (halves HBM traffic; rel-err gate is 2e-2, the fp16
    pipeline lands ~5e-4). Host transposes the fp16 result back.
  * Channels live on partitions, so per-channel tap weights are
    per-partition scalars and tap shifts are free-dim offsets.
  * Per [128, 512] chunk: PE accumulates taps 0..2 (shifted diag-matmuls,
    fp16 moving, fp32 PSUM accumulate). Tap 3 + bias either:
      - ACT computes a3 = w3*x + b, DVE STT adds it to PSUM  (default)
      - PE does a 4th matmul, ACT adds bias on the PSUM->SBUF copy
    CFG["n_pe_tap3"] picks how many of each group's 8 chunks use the PE
    path, to balance PE vs ACT vs DVE occupancy.
  * One whole-group-row DMA in/out ([128, 4096] fp16, 8 KiB/partition
    lines) amortizes DGE sequencer cost; l=0 zero-halo via 3-col memset
    on Pool (keeps the DVE queue free of the DMA ordering cycle).
  * Diagonal stationaries diag(w_k[g]) are host-prepared and DMA'd.
"""

import sys
import types

import numpy as np

try:  # the NTFF profile hook module is absent in some containers
    import antenv.axon_hooks  # noqa: F401
except Exception:
    _stub = types.ModuleType("antenv.axon_hooks")
    _stub.get_axon_ntff_profile_hook = lambda: None
    try:
        import antenv

        sys.modules["antenv.axon_hooks"] = _stub
        antenv.axon_hooks = _stub
    except Exception:
        _pkg = types.ModuleType("antenv")
        _pkg.axon_hooks = _stub
        sys.modules["antenv"] = _pkg
        sys.modules["antenv.axon_hooks"] = _stub

import concourse.bass as bass
import concourse.bacc as bacc
import concourse.mybir as mybir
from concourse.tile import TileContext
from concourse.bass_utils import run_bass_kernel_spmd

P = 128
B = 8
L = 4096
D = 1024
K = 4
G = D // P          # 8 channel groups
CL = 512            # l-chunk (one PSUM bank in fp32)
NCH = L // CL       # 8 chunks per group row

CFG = {
    "n_pe_tap3": 0,   # chunks per group where PE does tap3 (0..NCH)
    "xt_bufs": 1,
    "xt_slots": 8,    # all 8 input rows resident in SBUF
    "y_bufs": 1,
    "y_slots": 6,
    "a3_bufs": 8,
    "ps_bufs": 8,
    "in_splits": 2,   # x row-DMA split (fewer cols per DMA = earlier start)
    "out_splits": 2,  # y row-DMA split
    "out_eng": "sync",   # sync | scalar | gpsimd
    "const_eng": "gpsimd",
    "in_engs": ("sync",),
    "last_out_splits": 4,  # finer split for the last group's store (tail)
    "g0_in_splits": 4,     # finer split for the first group's load (startup)
    "act_warmup": True,    # hoist LoadActFuncSet before the main loop
    "dw_mode": "host",     # "host" (DMA'd) | "pool" (built on Pool engine)
    "pool_comb": (),       # chunk indices whose combine runs on Pool (unused:
                           # Pool ops reading PSUM do not lower in neuronxcc)
}

ALU = mybir.AluOpType


def _n_taps():
    return K if CFG["n_pe_tap3"] > 0 else K - 1


def _use_host_dw():
    return CFG["dw_mode"] == "host"


def build_conv_nc():
    f32 = mybir.dt.float32
    f16 = mybir.dt.float16
    NT = _n_taps()

    nc = bacc.Bacc("TRN2", target_bir_lowering=False)
    x_d = nc.dram_tensor("x", [D, L], f16, kind="ExternalInput")
    if _use_host_dw():
        dw_d = nc.dram_tensor("dw", [P, G * NT * P], f16, kind="ExternalInput")
    wcols_d = nc.dram_tensor("wcols", [P, G * K], f32, kind="ExternalInput")
    bcol_d = nc.dram_tensor("bcol", [P, G], f32, kind="ExternalInput")
    y_d = nc.dram_tensor("y", [D, L], f16, kind="ExternalOutput")

    n_pe = CFG["n_pe_tap3"]
    pe_t3 = {c for c in range(NCH) if (c * n_pe) // NCH != ((c + 1) * n_pe) // NCH}

    with TileContext(nc) as tc:
        with (
            tc.tile_pool(name="const", bufs=1) as constp,
            tc.tile_pool(name="xt", bufs=CFG["xt_bufs"]) as xtp,
            tc.tile_pool(name="a3", bufs=CFG["a3_bufs"]) as a3p,
            tc.tile_pool(name="yt", bufs=CFG["y_bufs"]) as ytp,
            tc.tile_pool(name="ps", bufs=CFG["ps_bufs"], space="PSUM") as psp,
        ):
            # consts ride a side DGE so SP's queue leads with x data;
            # group-0 stationaries first so PE can start ASAP
            c_eng = getattr(nc, CFG["const_eng"])
            dw = constp.tile([P, G * NT * P], f16)
            if _use_host_dw():
                c_eng.dma_start(out=dw[:, 0 : NT * P], in_=dw_d[:, 0 : NT * P])
            wcols = constp.tile([P, G * K], f32)
            c_eng.dma_start(out=wcols, in_=wcols_d[:, :])
            bcol = constp.tile([P, G], f32)
            c_eng.dma_start(out=bcol, in_=bcol_d[:, :])
            if _use_host_dw():
                for g in range(1, G):
                    c_eng.dma_start(
                        out=dw[:, g * NT * P : (g + 1) * NT * P],
                        in_=dw_d[:, g * NT * P : (g + 1) * NT * P],
                    )
            else:
                from concourse.masks import make_identity

                ident = constp.tile([P, P], f32)
                make_identity(nc, ident)
                for g in range(G):
                    for k in range(NT):
                        j = (g * NT + k) * P
                        nc.gpsimd.tensor_scalar_mul(
                            dw[:, j : j + P], ident[:, :],
                            wcols[:, g * K + k : g * K + k + 1],
                        )

            if CFG["act_warmup"]:
                warm = constp.tile([P, 1], f16)
                nc.scalar.activation(
                    warm[:, :], wcols[:, 0:1],
                    mybir.ActivationFunctionType.Identity,
                    bias=bcol[:, 0:1], scale=1.0,
                )

            def diag(g, k):
                j = (g * NT + k) * P
                return dw[:, j : j + P]

            # all input rows buffered up-front (fits SBUF at fp16); halo
            # memsets on DVE so no queue cycle with the const DMAs
            xts = []
            for g in range(G):
                n_is = CFG["in_splits"]
                if g == 0 and CFG["g0_in_splits"]:
                    n_is = CFG["g0_in_splits"]
                ISP = L // n_is
                x_row = x_d[g * P : (g + 1) * P, :]
                xt = xtp.tile([P, K - 1 + L], f16, tag=f"xt{g % CFG['xt_slots']}")
                nc.vector.memset(xt[:, 0 : K - 1], 0.0)
                for s in range(n_is):
                    i_eng = getattr(nc, CFG["in_engs"][(g * n_is + s)
                                                       % len(CFG["in_engs"])])
                    i_eng.dma_start(
                        out=xt[:, K - 1 + s * ISP : K - 1 + (s + 1) * ISP],
                        in_=x_row[:, s * ISP : (s + 1) * ISP],
                    )
                xts.append(xt)

            for g in range(G):
                xt = xts[g]
                y_row = y_d[g * P : (g + 1) * P, :]
                yt = ytp.tile([P, L], f16, tag=f"yt{g % CFG['y_slots']}")

                for c in range(NCH):
                    o = c * CL
                    ps = psp.tile([P, CL], f32)
                    use_pe = c in pe_t3
                    nk = K if use_pe else K - 1
                    for k in range(nk):
                        nc.tensor.matmul(
                            ps[:, :],
                            diag(g, k),
                            xt[:, o + k : o + k + CL],
                            start=(k == 0),
                            stop=(k == nk - 1),
                        )
                    if use_pe:
                        nc.scalar.activation(
                            yt[:, o : o + CL],
                            ps[:, :],
                            mybir.ActivationFunctionType.Identity,
                            bias=bcol[:, g : g + 1],
                            scale=1.0,
                        )
                    else:
                        a3 = a3p.tile([P, CL], f16)
                        nc.scalar.activation(
                            a3[:, :],
                            xt[:, o + K - 1 : o + K - 1 + CL],
                            mybir.ActivationFunctionType.Identity,
                            bias=bcol[:, g : g + 1],
                            scale=wcols[:, g * K + K - 1 : g * K + K],
                        )
                        if c in CFG["pool_comb"]:
                            nc.gpsimd.tensor_tensor(
                                out=yt[:, o : o + CL],
                                in0=a3[:, :],
                                in1=ps[:, :],
                                op=ALU.add,
                            )
                        else:
                            nc.vector.scalar_tensor_tensor(
                                out=yt[:, o : o + CL],
                                in0=a3[:, :],
                                scalar=1.0,
                                in1=ps[:, :],
                                op0=ALU.mult,
                                op1=ALU.add,
                            )
                n_os = CFG["out_splits"]
                if g == G - 1 and CFG["last_out_splits"]:
                    n_os = CFG["last_out_splits"]
                OSP = L // n_os
                o_eng = getattr(nc, CFG["out_eng"])
                for s in range(n_os):
                    o_eng.dma_start(
                        out=y_row[:, s * OSP : (s + 1) * OSP],
                        in_=yt[:, s * OSP : (s + 1) * OSP],
                    )
    nc.finalize()
    return nc


def host_prep(w, b):
    w = np.asarray(w, dtype=np.float32).reshape(D, K)
    b = np.asarray(b, dtype=np.float32).reshape(D)
    NT = _n_taps()
    wcols = np.empty((P, G * K), dtype=np.float32)
    bcol = np.empty((P, G), dtype=np.float32)
    rng = np.arange(P)
    for g in range(G):
        bcol[:, g] = b[g * P : (g + 1) * P]
        for k in range(K):
            wcols[:, g * K + k] = w[g * P : (g + 1) * P, k]
    out = {"wcols": wcols, "bcol": bcol}
    if _use_host_dw():
        dw = np.zeros((P, G * NT * P), dtype=np.float16)
        for g in range(G):
            for k in range(NT):
                dw[rng, (g * NT + k) * P + rng] = w[
                    g * P : (g + 1) * P, k
                ].astype(np.float16)
        out["dw"] = dw
    return out


_NC_CACHE = {}


def _get_nc():
    key = (L, D, CL, tuple(sorted(CFG.items())))
    if key not in _NC_CACHE:
        _NC_CACHE[key] = build_conv_nc()
    return _NC_CACHE[key]


def kernel(x, w, b, _trace=False):
    x = np.asarray(x, dtype=np.float32)
    assert x.shape == (B, L, D), x.shape
    consts = host_prep(w, b)
    nc = _get_nc()
    xt = np.ascontiguousarray(x.astype(np.float16).transpose(0, 2, 1))
    in_maps = [{"x": xt[i], **consts} for i in range(B)]
    res = run_bass_kernel_spmd(nc, in_maps, core_ids=list(range(B)), trace=_trace)
    y = np.stack(
        [res.results[i]["y"].T.astype(np.float32) for i in range(B)], axis=0
    )
    if _trace:
        return y, res
    return y


# revision 3
# speedup vs baseline: 1.0066x; 1.0066x over previous
"""Causal depthwise Conv1d (K=4) for Trainium2, 8 NeuronCores — v2.

Problem: x (B=8, L=4096, D=1024) f32, w (D, 1, 4), b (D,)
  y[n, l, d] = sum_k w[d, 0, k] * x[n, l - 3 + k, d] + b[d]   (zero pad l<0)

Sharding: data-parallel over batch — core i computes batch item i.

v2 strategy (vs v1's on-device PE transposes + fp32 exact):
  * Host pre-transposes x to channels-major (D, L) and casts to fp16;
    device I/O is fp16 (halves HBM traffic; rel-err gate is 2e-2, the fp16
    pipeline lands ~5e-4). Host transposes the fp16 result back.
  * Channels live on partitions, so per-channel tap weights are
    per-partition scalars and tap shifts are free-dim offsets.
  * Per [128, 512] chunk: PE accumulates taps 0..2 (shifted diag-matmuls,
    fp16 moving, fp32 PSUM accumulate). Tap 3 + bias either:
      - ACT computes a3 = w3*x + b, DVE STT adds it to PSUM  (default)
      - PE does a 4th matmul, ACT adds bias on the PSUM->SBUF copy
    CFG["n_pe_tap3"] picks how many of each group's 8 chunks use the PE
    path, to balance PE vs ACT vs DVE occupancy.
  * One whole-group-row DMA in/out ([128, 4096] fp16, 8 KiB/partition
    lines) amortizes DGE sequencer cost; l=0 zero-halo via 3-col memset
    on Pool (keeps the DVE queue free of the DMA ordering cycle).
  * Diagonal stationaries diag(w_k[g]) are host-prepared and DMA'd.
"""

import sys
import types

import numpy as np

try:  # the NTFF profile hook module is absent in some containers
    import antenv.axon_hooks  # noqa: F401
except Exception:
    _stub = types.ModuleType("antenv.axon_hooks")
    _stub.get_axon_ntff_profile_hook = lambda: None
    try:
        import antenv

        sys.modules["antenv.axon_hooks"] = _stub
        antenv.axon_hooks = _stub
    except Exception:
        _pkg = types.ModuleType("antenv")
        _pkg.axon_hooks = _stub
        sys.modules["antenv"] = _pkg
        sys.modules["antenv.axon_hooks"] = _stub

import concourse.bass as bass
import concourse.bacc as bacc
import concourse.mybir as mybir
from concourse.tile import TileContext
from concourse.bass_utils import run_bass_kernel_spmd

P = 128
B = 8
L = 4096
D = 1024
K = 4
G = D // P          # 8 channel groups
CL = 512            # l-chunk (one PSUM bank in fp32)
NCH = L // CL       # 8 chunks per group row

CFG = {
    "n_pe_tap3": 0,   # chunks per group where PE does tap3 (0..NCH)
    "xt_bufs": 1,
    "xt_slots": 8,    # all 8 input rows resident in SBUF
    "y_bufs": 1,
    "y_slots": 6,
    "a3_bufs": 8,
    "ps_bufs": 8,
    "in_splits": 2,   # x row-DMA split (fewer cols per DMA = earlier start)
    "out_splits": 2,  # y row-DMA split
    "out_eng": "sync",   # sync | scalar | gpsimd
    "const_eng": "gpsimd",
    "in_engs": ("sync",),
    "last_out_splits": 4,  # finer split for the last group's store (tail)
    "g0_in_splits": 4,     # finer split for the first group's load (startup)
    "g0_first_cols": 0,  # tiny chunk-0 prefetch for group 0
    "dw0_eng": "scalar",   # engine for group-0 stationaries (startup pole)
    "act_warmup": True,    # hoist LoadActFuncSet before the main loop
    "pe_warmup": 0,        # dummy matmuls to pre-ramp the PE p-state
    "dw_mode": "host",     # "host" (DMA'd) | "pool" (built on Pool engine)
    "pool_comb": (),       # chunk indices whose combine runs on Pool (unused:
                           # Pool ops reading PSUM do not lower in neuronxcc)
}

ALU = mybir.AluOpType


def _n_taps():
    return K if CFG["n_pe_tap3"] > 0 else K - 1


def _use_host_dw():
    return CFG["dw_mode"] == "host"


def build_conv_nc():
    f32 = mybir.dt.float32
    f16 = mybir.dt.float16
    NT = _n_taps()

    nc = bacc.Bacc("TRN2", target_bir_lowering=False)
    x_d = nc.dram_tensor("x", [D, L], f16, kind="ExternalInput")
    if _use_host_dw():
        dw_d = nc.dram_tensor("dw", [P, G * NT * P], f16, kind="ExternalInput")
    wcols_d = nc.dram_tensor("wcols", [P, G * K], f32, kind="ExternalInput")
    bcol_d = nc.dram_tensor("bcol", [P, G], f32, kind="ExternalInput")
    y_d = nc.dram_tensor("y", [D, L], f16, kind="ExternalOutput")

    n_pe = CFG["n_pe_tap3"]
    pe_t3 = {c for c in range(NCH) if (c * n_pe) // NCH != ((c + 1) * n_pe) // NCH}

    with TileContext(nc) as tc:
        with (
            tc.tile_pool(name="const", bufs=1) as constp,
            tc.tile_pool(name="xt", bufs=CFG["xt_bufs"]) as xtp,
            tc.tile_pool(name="a3", bufs=CFG["a3_bufs"]) as a3p,
            tc.tile_pool(name="yt", bufs=CFG["y_bufs"]) as ytp,
            tc.tile_pool(name="ps", bufs=CFG["ps_bufs"], space="PSUM") as psp,
        ):
            # consts ride a side DGE so SP's queue leads with x data;
            # group-0 stationaries first so PE can start ASAP
            c_eng = getattr(nc, CFG["const_eng"])
            dw = constp.tile([P, G * NT * P], f16)
            if _use_host_dw():
                d0_eng = getattr(nc, CFG["dw0_eng"])
                d0_eng.dma_start(out=dw[:, 0 : NT * P], in_=dw_d[:, 0 : NT * P])
            wcols = constp.tile([P, G * K], f32)
            c_eng.dma_start(out=wcols, in_=wcols_d[:, :])
            bcol = constp.tile([P, G], f32)
            c_eng.dma_start(out=bcol, in_=bcol_d[:, :])
            if _use_host_dw():
                for g in range(1, G):
                    c_eng.dma_start(
                        out=dw[:, g * NT * P : (g + 1) * NT * P],
                        in_=dw_d[:, g * NT * P : (g + 1) * NT * P],
                    )
            else:
                from concourse.masks import make_identity

                ident = constp.tile([P, P], f32)
                make_identity(nc, ident)
                for g in range(G):
                    for k in range(NT):
                        j = (g * NT + k) * P
                        nc.gpsimd.tensor_scalar_mul(
                            dw[:, j : j + P], ident[:, :],
                            wcols[:, g * K + k : g * K + k + 1],
                        )

            if CFG["act_warmup"]:
                warm = constp.tile([P, 1], f16)
                nc.scalar.activation(
                    warm[:, :], wcols[:, 0:1],
                    mybir.ActivationFunctionType.Identity,
                    bias=bcol[:, 0:1], scale=1.0,
                )

            def diag(g, k):
                j = (g * NT + k) * P
                return dw[:, j : j + P]

            if CFG["pe_warmup"]:
                ps_w = psp.tile([P, CL], f32, name="ps")
                for i in range(CFG["pe_warmup"]):
                    nc.tensor.matmul(
                        ps_w[:, 0 : NT * P], dw[:, 0:P], dw[:, 0 : NT * P],
                        start=True, stop=True,
                    )

            # all input rows buffered up-front (fits SBUF at fp16); halo
            # memsets on DVE so no queue cycle with the const DMAs
            xts = []
            for g in range(G):
                n_is = CFG["in_splits"]
                if g == 0 and CFG["g0_in_splits"]:
                    n_is = CFG["g0_in_splits"]
                ISP = L // n_is
                x_row = x_d[g * P : (g + 1) * P, :]
                xt = xtp.tile([P, K - 1 + L], f16, tag=f"xt{g % CFG['xt_slots']}")
                nc.vector.memset(xt[:, 0 : K - 1], 0.0)
                f0 = CFG["g0_first_cols"] if (g == 0 and CFG["g0_first_cols"]) else 0
                if f0:
                    nc.sync.dma_start(
                        out=xt[:, K - 1 : K - 1 + f0], in_=x_row[:, 0:f0]
                    )
                for s in range(n_is):
                    lo = max(s * ISP, f0)
                    hi = (s + 1) * ISP
                    if lo >= hi:
                        continue
                    i_eng = getattr(nc, CFG["in_engs"][(g * n_is + s)
                                                       % len(CFG["in_engs"])])
                    i_eng.dma_start(
                        out=xt[:, K - 1 + lo : K - 1 + hi],
                        in_=x_row[:, lo:hi],
                    )
                xts.append(xt)

            for g in range(G):
                xt = xts[g]
                y_row = y_d[g * P : (g + 1) * P, :]
                yt = ytp.tile([P, L], f16, tag=f"yt{g % CFG['y_slots']}")

                for c in range(NCH):
                    o = c * CL
                    ps = psp.tile([P, CL], f32)
                    use_pe = c in pe_t3
                    nk = K if use_pe else K - 1
                    for k in range(nk):
                        nc.tensor.matmul(
                            ps[:, :],
                            diag(g, k),
                            xt[:, o + k : o + k + CL],
                            start=(k == 0),
                            stop=(k == nk - 1),
                        )
                    if use_pe:
                        nc.scalar.activation(
                            yt[:, o : o + CL],
                            ps[:, :],
                            mybir.ActivationFunctionType.Identity,
                            bias=bcol[:, g : g + 1],
                            scale=1.0,
                        )
                    else:
                        a3 = a3p.tile([P, CL], f16)
                        nc.scalar.activation(
                            a3[:, :],
                            xt[:, o + K - 1 : o + K - 1 + CL],
                            mybir.ActivationFunctionType.Identity,
                            bias=bcol[:, g : g + 1],
                            scale=wcols[:, g * K + K - 1 : g * K + K],
                        )
                        if c in CFG["pool_comb"]:
                            nc.gpsimd.tensor_tensor(
                                out=yt[:, o : o + CL],
                                in0=a3[:, :],
                                in1=ps[:, :],
                                op=ALU.add,
                            )
                        else:
                            nc.vector.scalar_tensor_tensor(
                                out=yt[:, o : o + CL],
                                in0=a3[:, :],
                                scalar=1.0,
                                in1=ps[:, :],
                                op0=ALU.mult,
                                op1=ALU.add,
                            )
                n_os = CFG["out_splits"]
                if g == G - 1 and CFG["last_out_splits"]:
                    n_os = CFG["last_out_splits"]
                OSP = L // n_os
                o_eng = getattr(nc, CFG["out_eng"])
                for s in range(n_os):
                    o_eng.dma_start(
                        out=y_row[:, s * OSP : (s + 1) * OSP],
                        in_=yt[:, s * OSP : (s + 1) * OSP],
                    )
    nc.finalize()
    return nc


def host_prep(w, b):
    w = np.asarray(w, dtype=np.float32).reshape(D, K)
    b = np.asarray(b, dtype=np.float32).reshape(D)
    NT = _n_taps()
    wcols = np.empty((P, G * K), dtype=np.float32)
    bcol = np.empty((P, G), dtype=np.float32)
    rng = np.arange(P)
    for g in range(G):
        bcol[:, g] = b[g * P : (g + 1) * P]
        for k in range(K):
            wcols[:, g * K + k] = w[g * P : (g + 1) * P, k]
    out = {"wcols": wcols, "bcol": bcol}
    if _use_host_dw():
        dw = np.zeros((P, G * NT * P), dtype=np.float16)
        for g in range(G):
            for k in range(NT):
                dw[rng, (g * NT + k) * P + rng] = w[
                    g * P : (g + 1) * P, k
                ].astype(np.float16)
        out["dw"] = dw
    return out


_NC_CACHE = {}


def _get_nc():
    key = (L, D, CL, tuple(sorted(CFG.items())))
    if key not in _NC_CACHE:
        _NC_CACHE[key] = build_conv_nc()
    return _NC_CACHE[key]


def kernel(x, w, b, _trace=False):
    x = np.asarray(x, dtype=np.float32)
    assert x.shape == (B, L, D), x.shape
    consts = host_prep(w, b)
    nc = _get_nc()
    xt = np.ascontiguousarray(x.astype(np.float16).transpose(0, 2, 1))
    in_maps = [{"x": xt[i], **consts} for i in range(B)]
    res = run_bass_kernel_spmd(nc, in_maps, core_ids=list(range(B)), trace=_trace)
    y = np.stack(
        [res.results[i]["y"].T.astype(np.float32) for i in range(B)], axis=0
    )
    if _trace:
        return y, res
    return y
